# revision 1
# baseline (speedup 1.0000x reference)
"""Sparse (half-causal) multi-head attention on 8 Trainium2 NeuronCores.

Problem: x[2,2048,1024] -> QKV proj (16 heads, dk=dv=64) -> scores with
half-causal mask (rows <1024 attend cols <1024 dense; rows >=1024 causal)
-> softmax -> out proj.

Sharding: 8 cores = 2 batches x 4 head-groups (4 heads each).  Each core
computes its batch's full QKV for its 4 heads (column-sharded W), attention
for those heads, and a partial output projection (row-sharded Wo).  Host
sums the 4 partials per batch.

Per-core kernel design (all matmuls fp32r: full-rate, ~13-bit mantissa):
 - host feeds x^T so d_model lands on partitions for the projections
 - Q^T,K^T [256,2048] head-dim-on-partitions; V in natural [2048,64+1]
   layout with a ones column appended (denominator trick)
 - scores computed transposed, S^T[k,q] = K Q^T, so softmax sum over k is a
   matmul contraction: [V|1]^T P^T gives O^T stacked with the denominator
 - exp without max-subtraction (scores are O(1) by construction), 1/8 scale
   folded into the ACT activation scale
 - causal staircase handled by 4 precomputed [128,512] 0/1 mask tiles
 - biases folded in as K=1 rank-1 accumulating matmuls
"""

import sys

if "/opt/trn_rl_repo" not in sys.path:
    sys.path.insert(0, "/opt/trn_rl_repo")

import numpy as np

import concourse.bass as bass  # noqa: F401 (import registers engines)
import concourse.mybir as mybir
import concourse.tile as tile
from concourse import bacc
from concourse.bass_utils import run_bass_kernel_spmd

f32 = mybir.dt.float32
f32r = mybir.dt.float32r
AF = mybir.ActivationFunctionType
OP = mybir.AluOpType

D = 1024  # d_model
N = 2048  # n_ctx
HG = 256  # head-group width per core (4 heads x 64)


def make_tri() -> np.ndarray:
    """tri[kk, t, q'] = 1.0 if 128*t + kk <= q' else 0 — staircase masks."""
    kk = np.arange(128)[:, None, None]
    t = np.arange(4)[None, :, None]
    qp = np.arange(512)[None, None, :]
    return (128 * t + kk <= qp).astype(np.float32)


def build_nc():
    nc = bacc.Bacc("TRN2", target_bir_lowering=False, debug=False)

    xt = nc.declare_dram_parameter("xt", [D, N], f32r, isOutput=False)
    wq = nc.declare_dram_parameter("wq", [D, HG], f32r, isOutput=False)
    wk = nc.declare_dram_parameter("wk", [D, HG], f32r, isOutput=False)
    wv = nc.declare_dram_parameter("wv", [D, HG], f32r, isOutput=False)
    bqd = nc.declare_dram_parameter("bq", [HG], f32r, isOutput=False)
    bkd = nc.declare_dram_parameter("bk", [HG], f32r, isOutput=False)
    bvd = nc.declare_dram_parameter("bv", [HG], f32r, isOutput=False)
    wo = nc.declare_dram_parameter("wo", [HG, D], f32r, isOutput=False)
    trid = nc.declare_dram_parameter("tri", [128, 4, 512], f32, isOutput=False)
    onesd = nc.declare_dram_parameter("ones", [512], f32r, isOutput=False)
    y = nc.declare_dram_parameter("y", [N, D], f32, isOutput=True)
    y2 = nc.declare_dram_parameter("y2", [1024, D], f32, isOutput=True)

    dscr = nc.dram_tensor("dscr", [2, 2, 2, 1024], f32)  # (pair, parity, half)

    xt_r = xt[:].rearrange("(c p) n -> p c n", p=128)

    with tile.TileContext(nc) as tc:
        with (
            tc.tile_pool(name="persist", bufs=1) as P1,
            tc.tile_pool(name="xtp", bufs=2) as XTP,
            tc.tile_pool(name="ppool", bufs=5) as PP,
            tc.tile_pool(name="rp", bufs=2) as RP,
            tc.tile_pool(name="rbp", bufs=2) as RBP,
            tc.tile_pool(name="atp", bufs=3) as ATP,
            tc.tile_pool(name="yp", bufs=4) as YP,
            tc.tile_pool(name="ps_a", bufs=2, space="PSUM") as PSA,
            tc.tile_pool(name="ps_pv", bufs=2, space="PSUM") as PSPV,
            tc.tile_pool(name="ps_b", bufs=2, space="PSUM") as PSB,
        ):
            # ---------- constants / weights ----------
            wq_r = wq[:].rearrange("(c p) m -> p c m", p=128)
            wqa = P1.tile([128, 4, HG], f32r, tag="wqa")
            nc.sync.dma_start(wqa[:], wq_r[:, 0:4, :])
            wqb = P1.tile([128, 4, HG], f32r, tag="wqb")
            nc.sync.dma_start(wqb[:], wq_r[:, 4:8, :])
            wk_sb = P1.tile([128, 8, HG], f32r, tag="wk")
            wv_sb = P1.tile([128, 8, HG], f32r, tag="wv")
            wo_sb = P1.tile([128, 2, D], f32r, tag="wo")
            bq_sb = P1.tile([128, 2], f32, tag="bq")
            nc.gpsimd.dma_start(bq_sb[:], bqd[:].rearrange("(m p) -> p m", p=128))
            bk_sb = P1.tile([128, 2], f32, tag="bk")
            nc.gpsimd.dma_start(bk_sb[:], bkd[:].rearrange("(m p) -> p m", p=128))
            bv_sb = P1.tile([1, HG], f32r, tag="bv")
            nc.sync.dma_start(bv_sb[:], bvd[None, :])
            ones_sb = P1.tile([1, 512], f32r, tag="ones")
            nc.sync.dma_start(ones_sb[:], onesd[None, :])
            tri_sb = P1.tile([128, 4, 512], f32, tag="tri")

            qT = P1.tile([128, 2, N], f32r, tag="qT")
            kT = P1.tile([128, 2, N], f32r, tag="kT")
            v1 = P1.tile([128, 16, 4, 65], f32r, tag="v1")
            att = P1.tile([128, 2, N], f32r, tag="att")

            # ones column of [V|1] for the softmax denominator
            nc.sync.dma_start(v1[:, :, :, 64:65], onesd[0:64].partition_broadcast(128))

            # ---------- emitters ----------
            def emit_qkv_load(n4):
                ns = slice(512 * n4, 512 * n4 + 512)
                xt_n = (
                    XTP.tile([128, 4, 512], f32r, tag="xta", name=f"xta{n4}"),
                    XTP.tile([128, 4, 512], f32r, tag="xtb", name=f"xtb{n4}"),
                )
                nc.sync.dma_start(xt_n[0][:], xt_r[:, 0:4, ns])
                nc.sync.dma_start(xt_n[1][:], xt_r[:, 4:8, ns])
                return xt_n

            def emit_qkv_qk(n4, xt_n):
                """Q^T / K^T for one 512-wide seq chunk."""
                ns = slice(512 * n4, 512 * n4 + 512)
                for wget, bsb, dest in (
                    (lambda c, msl: (wqa if c < 4 else wqb)[:, c % 4, msl], bq_sb, qT),
                    (lambda c, msl: wk_sb[:, c, msl], bk_sb, kT),
                ):
                    for m in range(2):
                        msl = slice(128 * m, 128 * m + 128)
                        ps = PSB.tile([128, 512], f32, tag="b")
                        for c in range(8):
                            nc.tensor.matmul(
                                ps[:],
                                wget(c, msl),
                                xt_n[c // 4][:, c % 4, :],
                                start=(c == 0),
                                stop=(c == 7),
                            )
                        nc.vector.tensor_scalar_add(
                            dest[:, m, ns], ps[:], bsb[:, m : m + 1]
                        )

            def emit_qkv_v(n4, xt_n):
                """V natural rows for the 4 seq chunks of this n4."""
                for s in range(4 * n4, 4 * n4 + 4):
                    so = 128 * (s - 4 * n4)
                    ps = PSB.tile([128, 256], f32, tag="b")
                    for c in range(8):
                        nc.tensor.matmul(
                            ps[:],
                            xt_n[c // 4][:, c % 4, so : so + 128],
                            wv_sb[:, c, :],
                            start=(c == 0),
                            stop=False,
                        )
                    nc.tensor.matmul(
                        ps[:],
                        ones_sb[:, :128],
                        bv_sb[:],
                        start=False,
                        stop=True,
                    )
                    nc.vector.tensor_copy(
                        out=v1[:, s, :, 0:64],
                        in_=ps[:].rearrange("p (h d) -> p h d", h=4),
                    )

            def emit_qkv(n4):
                xt_n = emit_qkv_load(n4)
                emit_qkv_qk(n4, xt_n)
                emit_qkv_v(n4, xt_n)

            pv_tiles = {}

            def emit_attn_kc(hp, half, par, kc_lo, kc_hi):
                """Scores+exp+PV for one head parity over k-chunks [kc_lo, kc_hi)."""
                q0 = 1024 * half
                seg_last = [7, 7] if half == 0 else [11, 15]
                if kc_lo == 0:
                    pv_tiles[(hp, half, par)] = [
                        PSPV.tile([65, 512], f32, tag="pv", name=f"pv{hp}{half}{par}{i}")
                        for i in range(2)
                    ]
                pv = pv_tiles[(hp, half, par)]
                base = 64 * par
                for kc in range(kc_lo, kc_hi):
                    diag = half == 1 and kc >= 8
                    vq = 128 * (kc - 8) if diag else 0
                    segs = [nn for nn in range(2) if 512 * nn + 512 > vq]
                    s_t = PSA.tile(
                        [128, 1024], f32, tag="s", name=f"s{hp}{half}{par}{kc}"
                    )
                    for nn in segs:
                        qs = slice(q0 + 512 * nn, q0 + 512 * nn + 512)
                        nc.tensor.matmul(
                            s_t[:, 512 * nn : 512 * nn + 512],
                            kT[base : base + 64, hp, 128 * kc : 128 * kc + 128],
                            qT[base : base + 64, hp, qs],
                            start=True,
                            stop=True,
                        )
                    p_t = PP.tile([128, 1024], f32r, tag="p")
                    if not diag:
                        nc.scalar.activation(p_t[:], s_t[:], AF.Exp, scale=0.125)
                    else:
                        # one exp over the valid segs, then mask the diagonal
                        # seg in place (reading the f32r tile as f32 bits)
                        mseg = vq // 512
                        t = (vq - 512 * mseg) // 128
                        lo = 512 * segs[0]
                        nc.scalar.activation(
                            p_t[:, lo:1024], s_t[:, lo:1024], AF.Exp, scale=0.125
                        )
                        msl = slice(512 * mseg, 512 * mseg + 512)
                        nc.vector.tensor_tensor(
                            p_t[:, msl],
                            p_t[:, msl].bitcast(f32),
                            tri_sb[:, t, :],
                            OP.mult,
                        )
                    # PV accumulation (+ denominator row 64)
                    for nn in segs:
                        sl_ = slice(512 * nn, 512 * nn + 512)
                        nc.tensor.matmul(
                            pv[nn][0:65, :],
                            v1[:, kc, 2 * hp + par, :],
                            p_t[:, sl_],
                            start=(kc == 0),
                            stop=(kc == seg_last[nn]),
                        )

            def emit_attn_norm(hp, half, par, seg, fast=False):
                """Normalize one 512-wide q seg: att = O^T * (1/denom).  Stage
                through SBUF so the pv bank frees without waiting the denom
                broadcast.  fast=True broadcasts via a K=1 matmul into PSUM
                (no DRAM roundtrip) — used where the PE is otherwise idle."""
                q0 = 1024 * half + 512 * seg
                pv = pv_tiles[(hp, half, par)]
                sl = slice(512 * seg, 512 * seg + 512)
                stage = ATP.tile([65, 512], f32, tag="at", name=f"st{hp}{half}{par}{seg}")
                nc.vector.tensor_copy(out=stage[:], in_=pv[seg][:, :])
                if fast:
                    rr = RP.tile([1, 512], f32r, tag="r", name=f"rf{hp}{half}{par}{seg}")
                    with nc.allow_low_precision(reason="f32r denom for K=1 broadcast"):
                        nc.vector.reciprocal(rr[:], stage[64:65, :])
                    rb = PSB.tile([64, 512], f32, tag="b", name=f"rbp{hp}{half}{par}{seg}")
                    nc.tensor.matmul(rb[:], ones_sb[:, :64], rr[:], start=True, stop=True)
                else:
                    r_sb = RP.tile([1, 512], f32, tag="r", name=f"r{hp}{half}{par}{seg}")
                    nc.vector.reciprocal(r_sb[:], stage[64:65, :])
                    nc.sync.dma_start(dscr[hp, par, half, sl], r_sb[:])
                    rb = RBP.tile([64, 512], f32, tag="rb", name=f"rb{hp}{half}{par}{seg}")
                    nc.sync.dma_start(
                        rb[:], dscr[hp, par, half, sl].partition_broadcast(64)
                    )
                base = 64 * par  # DVE partition-offset write for par 1
                nc.vector.tensor_tensor(
                    att[base : base + 64, hp, q0 : q0 + 512], stage[0:64, :], rb[:], OP.mult
                )

            def emit_outproj(s_lo, s_hi, act_copies=False):
                """Partial output projection for seq chunks [s_lo, s_hi)."""
                for s in range(s_lo, s_hi):
                    yt = YP.tile([128, D], f32, tag="y", name=f"yt{s}")
                    for nseg in range(2):
                        ps = PSB.tile([128, 512], f32, tag="b", name=f"yps{s}{nseg}")
                        for hp in range(2):
                            nc.tensor.matmul(
                                ps[:],
                                att[:, hp, 128 * s : 128 * s + 128],
                                wo_sb[:, hp, 512 * nseg : 512 * nseg + 512],
                                start=(hp == 0),
                                stop=(hp == 1),
                            )
                        dst = yt[:, 512 * nseg : 512 * nseg + 512]
                        if act_copies:
                            nc.scalar.copy(out=dst, in_=ps[:])
                        else:
                            nc.vector.tensor_copy(out=dst, in_=ps[:])
                        nc.sync.dma_start(
                            y[128 * s : 128 * s + 128, 512 * nseg : 512 * nseg + 512],
                            dst,
                        )

            def emit_outproj_hp(s_lo, s_hi, hp, act_copies=False):
                """Single-head-pair out-proj pass; hp=1 accumulates into y."""
                for s in range(s_lo, s_hi):
                    yt = YP.tile([128, D], f32, tag="y", name=f"yth{s}{hp}")
                    for nseg in range(2):
                        ps = PSB.tile([128, 512], f32, tag="b", name=f"ypsh{s}{nseg}{hp}")
                        nc.tensor.matmul(
                            ps[:],
                            att[:, hp, 128 * s : 128 * s + 128],
                            wo_sb[:, hp, 512 * nseg : 512 * nseg + 512],
                            start=True,
                            stop=True,
                        )
                        dst = yt[:, 512 * nseg : 512 * nseg + 512]
                        if act_copies:
                            nc.scalar.copy(out=dst, in_=ps[:])
                        else:
                            nc.vector.tensor_copy(out=dst, in_=ps[:])
                        tgt = (
                            y[128 * s : 128 * s + 128, 512 * nseg : 512 * nseg + 512]
                            if hp == 0
                            else y2[
                                128 * (s - 8) : 128 * (s - 8) + 128,
                                512 * nseg : 512 * nseg + 512,
                            ]
                        )
                        nc.sync.dma_start(tgt, dst)

            # ---------- emission order: overlap QKV/outproj PE work with exp-bound attention ----------
            xt0 = emit_qkv_load(0)
            nc.sync.dma_start(wk_sb[:], wk[:].rearrange("(c p) m -> p c m", p=128))
            nc.sync.dma_start(wv_sb[:], wv[:].rearrange("(c p) m -> p c m", p=128))
            emit_qkv_qk(0, xt0)
            emit_qkv_v(0, xt0)
            emit_qkv(1)
            # q-half 0 of both head pairs only needs xt chunks 0-1
            for par in range(2):
                emit_attn_kc(0, 0, par, 0, 8)
                emit_attn_norm(0, 0, par, 0)
                emit_attn_norm(0, 0, par, 1)
            # non-critical loads go here: the ramp is DMA-bandwidth-bound and
            # these 6MB would delay xt1/wk/wv; DMA idles during attention
            nc.sync.dma_start(tri_sb[:], trid[:])
            nc.sync.dma_start(wo_sb[:], wo[:].rearrange("(c p) n -> p c n", p=128))
            xt2 = emit_qkv_load(2)
            xt3 = emit_qkv_load(3)
            for par in range(2):
                emit_attn_kc(1, 0, par, 0, 8)
                emit_attn_norm(1, 0, par, 0)
                emit_attn_norm(1, 0, par, 1)
            # chunk 2/3 projections gate only the half-1 attention blocks:
            # emitted after the half-0 blocks they become pure PE filler
            emit_qkv_qk(2, xt2)
            emit_qkv_qk(3, xt3)
            emit_qkv_v(2, xt2)
            emit_qkv_v(3, xt3)
            for par in range(2):
                emit_attn_kc(0, 1, par, 0, 12)
                emit_attn_norm(0, 1, par, 0)
                emit_attn_kc(0, 1, par, 12, 16)
                emit_attn_norm(0, 1, par, 1)
            # rows [0, 1024) of the output only need q-half 0 attention
            emit_outproj(0, 8)
            emit_attn_kc(1, 1, 0, 0, 12)
            emit_attn_norm(1, 1, 0, 0)
            emit_attn_kc(1, 1, 0, 12, 16)
            emit_attn_norm(1, 1, 0, 1)
            emit_attn_kc(1, 1, 1, 0, 12)
            emit_attn_norm(1, 1, 1, 0)
            emit_outproj_hp(8, 16, 0)
            emit_attn_kc(1, 1, 1, 12, 16)
            # seq chunks 8-11 only need q-seg 0 of the last block
            emit_outproj_hp(8, 12, 1, act_copies=True)
            emit_attn_norm(1, 1, 1, 1)
            emit_outproj_hp(12, 16, 1, act_copies=True)

    nc.compile()
    return nc


_NC = None
_TRI = None
_ONES = None


def _get_nc():
    global _NC, _TRI, _ONES
    if _NC is None:
        _NC = build_nc()
        _TRI = make_tri()
        _ONES = np.ones(512, np.float32)
    return _NC


def make_in_maps(x, Wq, bq, Wk, bk, Wv, bv, Wo):
    _get_nc()
    x = np.asarray(x, np.float32)
    in_maps = []
    for core in range(8):
        b, g = core // 4, core % 4
        sl = slice(HG * g, HG * (g + 1))
        in_maps.append(
            {
                "xt": np.ascontiguousarray(x[b].T),
                "wq": np.ascontiguousarray(np.asarray(Wq, np.float32)[:, sl]),
                "wk": np.ascontiguousarray(np.asarray(Wk, np.float32)[:, sl]),
                "wv": np.ascontiguousarray(np.asarray(Wv, np.float32)[:, sl]),
                "bq": np.ascontiguousarray(np.asarray(bq, np.float32)[sl]),
                "bk": np.ascontiguousarray(np.asarray(bk, np.float32)[sl]),
                "bv": np.ascontiguousarray(np.asarray(bv, np.float32)[sl]),
                "wo": np.ascontiguousarray(np.asarray(Wo, np.float32)[sl, :]),
                "tri": _TRI,
                "ones": _ONES,
            }
        )
    return in_maps


def kernel(x, Wq, bq, Wk, bk, Wv, bv, Wo, _trace=False, _trace_kwargs=None):
    nc = _get_nc()
    in_maps = make_in_maps(x, Wq, bq, Wk, bk, Wv, bv, Wo)
    res = run_bass_kernel_spmd(
        nc, in_maps, list(range(8)), trace=_trace, **(_trace_kwargs or {})
    )
    out = np.zeros((2, N, D), np.float64)
    for core in range(8):
        out[core // 4] += res.results[core]["y"].astype(np.float64)
        out[core // 4, 1024:] += res.results[core]["y2"].astype(np.float64)
    y = out.astype(np.float32)
    if _trace:
        return y, res
    return y

